# revision 2
# baseline (speedup 1.0000x reference)
"""Trainium2 Bass kernel for nn_Attention_82781199663345 (sparse_attention).

Reference computation (see problem statement):
    q  = x @ Wq.T + bq                    -> heads interleaved: head n owns q[i*8+n]
    K  = (memory @ Wk.T + bk)             -> (L, H), same interleave
    QK[n,l] = (d**-.5) * sum_i q[i*8+n] * K[l, i*8+n]
    attn = softmax_l(QK)                  (pad-mask term is exactly 0.0 in fp32)
    V  = memory @ Wv.T + bv
    feat[n,i] = sum_l attn[n,l] * V[l, i*8+n]
    out = relu(concat(x, feat) @ Wo.T + bo)

Algebraic refactor (exact in real arithmetic):
  * QK[n,l] = memory[l] . w_n + c_n   with  w_n = sum_i q_s[i*8+n] * Wk[i*8+n, :]
    (c_n is constant per head -> cancels in softmax, dropped)
  * sum_l attn[n,l] = 1  =>  feat row n = (attn[n] @ memory) @ Wv.T + bv, sliced
    at columns i*8+n.
  So the only L-sized (memory-bound) work is:
      scores = memory @ W            (L, 8)
      ctx    = softmax(scores).T @ memory   (8, 2048)
  Everything else is O(H*MD) and done on host in fp32.

Device strategy (8 cores, sequence-parallel over L):
  Each core gets its 2048-row shard twice in fp8e4m3: pre-transposed (d,l)
  for the scores pass and natural (l,d) for the context pass (the PE
  contracts over the partition dim only).  Softmax uses no max-subtraction:
  the final ctx/s division cancels any constant factor, and scores are
  O(+-2.5) so exp(scores) is far from fp16 overflow.  The cross-core
  combine is a pure sum on host: ctx = sum_c ctx_c, s = sum_c s_c.

DMA layout: both mem copies are packed host-side so each DMA instruction
moves [128, N] with N contiguous bytes per partition (8 KiB rows ->
128 descriptors per MiB, half the descriptor-generation cost of the
2 KiB-line layout).  memT streams first (4 x 1 MiB on the sync HWDGE
queue), then memn with a tapered tail (1 MiB groups down to a single
256 KiB tile) so the last-arriving bytes gate minimal remaining work.
Small operands (wt, eye) and the s/ctx_hi outputs ride the scalar-engine
HWDGE queue so they never stall the input stream.
"""

import sys

import numpy as np

if "/opt/trn_rl_repo" not in sys.path:
    sys.path.insert(0, "/opt/trn_rl_repo")

H = 1024          # hidden dim
MD = 2048         # memory dim
L = 16384         # memory length
NH = 8            # heads
NCORES = 8
LSH = L // NCORES         # 2048 rows per core
DHEAD = H // NH           # 128
DC = MD // 128            # 16 contraction chunks (scores pass)
LT = LSH // 128           # 16 l-tiles (context pass)
MEMT_GRP = 4              # memT chunks per DMA (1 MiB)
MEMN_GRPS = (4, 4, 4, 2, 1, 1)   # memn l-tiles per DMA, tapered tail

_CACHE = {}


def _build_nc():
    import concourse.bass as bass
    import concourse.mybir as mybir
    from concourse import tile

    fp16 = mybir.dt.float16
    fp8 = mybir.dt.float8e4
    f32 = mybir.dt.float32
    Exp = mybir.ActivationFunctionType.Exp

    nc = bass.Bass()
    # Bass.__init__ ends with four Pool-engine const memsets and an
    # all-engine barrier.  The barrier costs ~3.4us of kernel time because
    # every engine waits for the slow Q7 memsets before starting; nothing
    # here consumes those consts (the exp bias is built on ACT), so drop
    # the barrier (keep the memsets) and let the DMA stream start
    # immediately.
    preamble_barrier = [
        i.name
        for f in nc.m.functions
        for b in f.blocks
        for i in b.instructions
        if isinstance(i, (mybir.InstDrain, mybir.InstEventSemaphore))
    ]
    memT_d = nc.dram_tensor("memT", [128, DC * LSH], fp8, kind="ExternalInput")
    memn_d = nc.dram_tensor("memn", [128, LT * MD], fp8, kind="ExternalInput")
    wt_d = nc.dram_tensor("wt", [128, DC * NH], fp16, kind="ExternalInput")
    ctx_d = nc.dram_tensor("ctx", [NH, MD], f32, kind="ExternalOutput")
    s_d = nc.dram_tensor("s", [NH, 2], f32, kind="ExternalOutput")
    eye_np = np.zeros((NH, NH), dtype=np.float16)
    eye_np[:NH, :NH] = np.eye(NH, dtype=np.float16)
    eye_d = nc.inline_tensor(eye_np, "eye8")

    with tile.TileContext(nc) as tc:
        with (
            tc.tile_pool(name="const", bufs=1) as constp,
            tc.tile_pool(name="memTp", bufs=1) as memTp,
            tc.tile_pool(name="memnp", bufs=1) as memnp,
            tc.tile_pool(name="small", bufs=1) as smallp,
            tc.tile_pool(name="pssc", bufs=1, space=bass.MemorySpace.PSUM) as pssc,
            tc.tile_pool(name="pstr", bufs=1, space=bass.MemorySpace.PSUM) as pstr,
        ):
            # Small operands on the scalar (ACT) HWDGE queue: they land
            # before the first memT group and never contend with the
            # sync-queue input stream.
            wt_sb = constp.tile([128, DC * NH], fp16, tag="wt")
            nc.scalar.dma_start(out=wt_sb[:], in_=wt_d[:])
            eye_sb = constp.tile([NH, NH], fp16, tag="eye")
            nc.scalar.dma_start(out=eye_sb[:], in_=eye_d[:])

            # Input stream on the sync HWDGE queue, memT strictly first
            # (pass A must finish before pass B's operands are useful).
            memT_sb = []
            for g in range(DC // MEMT_GRP):
                t_ = memTp.tile([128, MEMT_GRP * LSH], fp8, tag=f"memT{g}")
                nc.sync.dma_start(
                    out=t_[:],
                    in_=memT_d[:, g * MEMT_GRP * LSH : (g + 1) * MEMT_GRP * LSH],
                )
                memT_sb.append(t_)

            def memT_chunk(c, nb):
                g, j = c // MEMT_GRP, c % MEMT_GRP
                off = j * LSH + nb * 512
                return memT_sb[g][:, off : off + 512]

            memn_sb = []
            memn_start = []
            pos = 0
            for k, gsz in enumerate(MEMN_GRPS):
                t_ = memnp.tile([128, gsz * MD], fp8, tag=f"memn{k}")
                nc.sync.dma_start(
                    out=t_[:], in_=memn_d[:, pos * MD : (pos + gsz) * MD]
                )
                memn_sb.append(t_)
                memn_start.append(pos)
                pos += gsz

            def memn_tile(t, q):
                for k in range(len(MEMN_GRPS) - 1, -1, -1):
                    if t >= memn_start[k]:
                        off = (t - memn_start[k]) * MD + q * 512
                        return memn_sb[k][:, off : off + 512]
                raise AssertionError

            # Pass A: scoresT[n, l] = sum_d w[d, n] * memT[d, l], accumulated
            # over 16 d-chunks into 2 two-bank psum tiles (c outer so
            # accumulation chases the DMA arrivals).  All four l-block
            # chains write rows 0:8 in PE column group 0 so each psum tile
            # drains with a single wide instruction.
            scA = pssc.tile([NH, 1024], f32, tag="scA")
            scB = pssc.tile([NH, 1024], f32, tag="scB")
            sc_out = [
                scA[:, 0:512], scA[:, 512:1024],
                scB[:, 0:512], scB[:, 512:1024],
            ]
            for c in range(DC):
                for nb in range(4):
                    nc.tensor.matmul(
                        sc_out[nb],
                        wt_sb[:, c * NH : (c + 1) * NH],
                        memT_chunk(c, nb),
                        start=(c == 0),
                        stop=(c == DC - 1),
                    )

            # p = exp(scores); accum_out gives the softmax partial sum.  No
            # max-subtraction needed (see module docstring).  The zero bias
            # is built on ACT itself (wt * 0.0) so nothing depends on the
            # stripped init barrier.
            zero_b = constp.tile([128, 1], f32, tag="zerob")
            nc.scalar.mul(zero_b[:], wt_sb[:, 0:1], 0.0)
            pT_sb = smallp.tile([NH, 2 * 1024], fp16, tag="pT")
            s_sb = smallp.tile([NH, 2], f32, tag="s")
            nc.scalar.activation(
                pT_sb[:, 0:1024], scA[:], Exp, bias=zero_b[0:NH, :],
                scale=1.0, accum_out=s_sb[:, 0:1],
            )
            nc.scalar.activation(
                pT_sb[:, 1024:2048], scB[:], Exp, bias=zero_b[0:NH, :],
                scale=1.0, accum_out=s_sb[:, 1:2],
            )
            # Ship s mid-stream on the scalar queue: it is final as soon as
            # the exps ran, and it must not stall the sync input stream.
            nc.scalar.dma_start(out=s_d[:], in_=s_sb[:])

            # Transpose p (8, L) -> per-l-tile (128, 8) stationary operands.
            tr_ps = pstr.tile([128, LT * NH], fp16, tag="tr")
            for t in range(LT):
                nc.tensor.transpose(
                    tr_ps[:, t * NH : (t + 1) * NH],
                    pT_sb[:, t * 128 : (t + 1) * 128],
                    eye_sb[:],
                )
            p_all = smallp.tile([128, LT * NH], fp16, tag="pall")
            nc.vector.tensor_copy(p_all[:], tr_ps[:])

            # Pass B: ctx[n, d] = sum_l p[l, n] * mem[l, d], accumulated over
            # 16 l-tiles (t outer: rides the memn DMAs).  Reuses the scores
            # psum tiles — the WAR dependency on the exps is long satisfied
            # by the time the first memn group lands.
            cx_out = sc_out
            for t in range(LT):
                for q in range(4):
                    nc.tensor.matmul(
                        cx_out[q],
                        p_all[:, t * NH : (t + 1) * NH],
                        memn_tile(t, q),
                        start=(t == 0),
                        stop=(t == LT - 1),
                    )

            # Drain ctx to SBUF with ACT and DVE in parallel (one wide copy
            # each), then ship on both HWDGE queues.
            ctx_lo = smallp.tile([NH, 1024], f32, tag="ctxlo")
            ctx_hi = smallp.tile([NH, 1024], f32, tag="ctxhi")
            nc.scalar.copy(ctx_lo[:], scA[:])
            nc.vector.tensor_copy(ctx_hi[:], scB[:])
            nc.sync.dma_start(out=ctx_d[:, 0:1024], in_=ctx_lo[:])
            nc.scalar.dma_start(out=ctx_d[:, 1024:], in_=ctx_hi[:])

    names = set(preamble_barrier)
    for f in nc.m.functions:
        for b in f.blocks:
            insts = b.instructions
            keep = [i for i in insts if i.name not in names]
            if len(keep) != len(insts):
                insts[:] = keep

    _split_multiwait(nc, mybir)
    nc.finalize()
    return nc


def _split_multiwait(nc, mybir):
    """Split instructions carrying >1 semaphore wait into single-wait NoOps.

    The walrus build in this environment encodes exactly one sync wait per
    engine instruction (setupSyncWait raises "Too many sync wait commands"
    otherwise), but Tile attaches the full wait set of the kernel-tail drain
    to one instruction.  Hoist all but the last wait onto dedicated NoOps on
    the same engine queue, which preserves semantics exactly.
    """
    k = 0
    for func in nc.m.functions:
        for block in func.blocks:
            insts = block.instructions
            i = 0
            while i < len(insts):
                inst = insts[i]
                si = inst.sync_info
                if si is not None and si.on_wait and len(si.on_wait) > 1:
                    waits = list(si.on_wait)
                    nops = []
                    for w in waits[:-1]:
                        nop = mybir.InstNoOp(
                            name=f"I-waitsplit-{k}",
                            engine=inst.engine,
                            bass_nofuse=True,
                            sync_info=mybir.SyncInfo(on_wait=[w], on_update=[]),
                        )
                        k += 1
                        nc.register_instruction(nop)
                        nops.append(nop)
                    inst.sync_info = mybir.SyncInfo(
                        on_wait=[waits[-1]], on_update=list(si.on_update)
                    )
                    insts[i:i] = nops
                    i += len(nops)
                i += 1


def _get_nc():
    if "nc" not in _CACHE:
        _CACHE["nc"] = _build_nc()
    return _CACHE["nc"]


def _host_prep(inputs):
    x = np.asarray(inputs["x"], dtype=np.float32).reshape(-1)          # (1024,)
    memory = np.asarray(inputs["memory"], dtype=np.float32)            # (L, MD)
    Wq = np.asarray(inputs["Wq"], dtype=np.float32)
    bq = np.asarray(inputs["bq"], dtype=np.float32)
    Wk = np.asarray(inputs["Wk"], dtype=np.float32)

    q = (x @ Wq.T + bq) * (DHEAD ** -0.5)                              # (1024,)
    # w[:, n] = sum_i q[i*8+n] * Wk[i*8+n, :]
    wmat = np.einsum(
        "in,ind->dn", q.reshape(DHEAD, NH), Wk.reshape(DHEAD, NH, MD),
        optimize=True,
    ).astype(np.float32)                                               # (MD, 8)
    wt_packed = np.ascontiguousarray(
        wmat.reshape(DC, 128, NH).transpose(1, 0, 2).reshape(128, DC * NH)
    ).astype(np.float16)

    import ml_dtypes
    fp8 = ml_dtypes.float8_e4m3
    in_maps = []
    for c in range(NCORES):
        shard = memory[c * LSH : (c + 1) * LSH]                        # (LSH, MD)
        # memT packed: [p, c*LSH + l] = shard[l, c*128+p]
        mt = shard.T.astype(fp8)                                       # (MD, LSH)
        memT_pack = np.ascontiguousarray(
            mt.reshape(DC, 128, LSH).transpose(1, 0, 2).reshape(128, DC * LSH)
        )
        # memn packed: [p, t*MD + d] = shard[t*128+p, d]
        mn = shard.astype(fp8)                                         # (LSH, MD)
        memn_pack = np.ascontiguousarray(
            mn.reshape(LT, 128, MD).transpose(1, 0, 2).reshape(128, LT * MD)
        )
        in_maps.append(
            {"memT": memT_pack, "memn": memn_pack, "wt": wt_packed}
        )
    return in_maps


def _host_finish(inputs, ctx_tot, s_tot):
    x = np.asarray(inputs["x"], dtype=np.float32).reshape(-1)
    Wv = np.asarray(inputs["Wv"], dtype=np.float32)
    bv = np.asarray(inputs["bv"], dtype=np.float32)
    Wo = np.asarray(inputs["Wo"], dtype=np.float32)
    bo = np.asarray(inputs["bo"], dtype=np.float32)

    ctx_norm = ctx_tot / s_tot                                         # (8, MD)
    feat_full = ctx_norm @ Wv.T + bv                                   # (8, 1024)
    feat = np.empty(H, dtype=np.float32)
    for n in range(NH):
        feat[n::NH] = feat_full[n, n::NH]
    ax = np.concatenate([x, feat])
    out = np.maximum(ax @ Wo.T + bo, 0.0).astype(np.float32)
    return out.reshape(1, 1, H)


def _run(inputs, trace=False, **spmd_kwargs):
    from concourse.bass_utils import run_bass_kernel_spmd

    nc = _get_nc()
    in_maps = _host_prep(inputs)
    res = run_bass_kernel_spmd(
        nc, in_maps, list(range(NCORES)), trace=trace, **spmd_kwargs
    )
    ctx_tot = np.zeros((NH, MD), dtype=np.float32)
    s_tot = np.zeros((NH, 1), dtype=np.float32)
    for r in res.results:
        ctx_tot += r["ctx"].astype(np.float32)
        s_tot += r["s"].astype(np.float32).sum(axis=1, keepdims=True)
    return _host_finish(inputs, ctx_tot, s_tot), res


def kernel(**inputs) -> np.ndarray:
    out, _ = _run(inputs, trace=False)
    return out


# revision 17
# speedup vs baseline: 1.2247x; 1.2247x over previous
"""Trainium2 Bass kernel for nn_Attention_82781199663345 (sparse_attention).

Reference computation (see problem statement):
    q  = x @ Wq.T + bq                    -> heads interleaved: head n owns q[i*8+n]
    K  = (memory @ Wk.T + bk)             -> (L, H), same interleave
    QK[n,l] = (d**-.5) * sum_i q[i*8+n] * K[l, i*8+n]
    attn = softmax_l(QK)                  (pad-mask term is exactly 0.0 in fp32)
    V  = memory @ Wv.T + bv
    feat[n,i] = sum_l attn[n,l] * V[l, i*8+n]
    out = relu(concat(x, feat) @ Wo.T + bo)

Algebraic refactor (exact in real arithmetic):
  * QK[n,l] = memory[l] . w_n + c_n   with  w_n = sum_i q_s[i*8+n] * Wk[i*8+n, :]
    (c_n is constant per head -> cancels in softmax, dropped)
  * sum_l attn[n,l] = 1  =>  feat row n = (attn[n] @ memory) @ Wv.T + bv, sliced
    at columns i*8+n.
  So the only L-sized (memory-bound) work is:
      scores = memory @ W            (L, 8)
      ctx    = softmax(scores).T @ memory   (8, 2048)
  Everything else is O(H*MD) and done on host in fp32.

Device strategy (8 cores, sequence-parallel over L):
  Each core gets its 2048-row shard twice in fp8e4m3: pre-transposed (d,l)
  for the scores pass and natural (l,d) for the context pass (the PE
  contracts over the partition dim only).  Softmax uses no max-subtraction:
  the final ctx/s division cancels any constant factor, and scores are
  O(+-2.5) so exp(scores) is far from fp16 overflow.  The cross-core
  combine is a pure sum on host: ctx = sum_c ctx_c, s = sum_c s_c.

Performance structure (from trace analysis):
  * Host packs both mem copies so every DMA moves [128, N] with N
    contiguous bytes per partition (8 KiB rows -> 128 descriptors/MiB);
    this sustains ~420 GB/s per core vs ~300 with 2 KiB lines.
  * All input DMAs ride the sync HWDGE queue (memT first, memn tapered);
    wt/eye/s/ctx_hi ride the scalar queue so they never stall the stream.
  * Matmuls: fp8e4 DoubleRow perf mode (256-deep contraction per pass),
    wt prescaled by 256 into fp8 (compensated in the exp scale), attention
    weights quantized to fp8 for the context pass.
  * Two PE column groups (rows 0:8 / 32:40), matmul order alternates
    groups: back-to-back same-group matmuls serialize (~227 ns/512 cols),
    alternating groups pipeline (~117 ns).
  * Scores psum is two 2-bank tiles, so softmax is 4 narrow exps whose
    band transposes pipeline on the PE while the next exp runs on ACT.
"""

import sys

import numpy as np

if "/opt/trn_rl_repo" not in sys.path:
    sys.path.insert(0, "/opt/trn_rl_repo")

H = 1024          # hidden dim
MD = 2048         # memory dim
L = 16384         # memory length
NH = 8            # heads
NCORES = 8
LSH = L // NCORES         # 2048 rows per core
DHEAD = H // NH           # 128
DC = MD // 128            # 16 contraction chunks (scores pass)
LT = LSH // 128           # 16 l-tiles (context pass)
WT_SCALE = 256.0          # wt prescale so fp8e4m3 stays in normal range
MEMT_GRPS = (2, 2, 4, 4, 4)      # memT d-chunks per DMA (pair-aligned)
MEMN_GRPS = (4, 4, 4, 2)         # memn l-tiles per DMA; tiles 14/15 are a
                                 # split pair (two 256 KiB DMAs, one tile)

_CACHE = {}


def _build_nc():
    import concourse.bass as bass
    import concourse.mybir as mybir
    from concourse import tile

    fp16 = mybir.dt.float16
    fp8 = mybir.dt.float8e4
    f32 = mybir.dt.float32
    Exp = mybir.ActivationFunctionType.Exp
    DR = mybir.MatmulPerfMode.DoubleRow

    nc = bass.Bass()
    # Bass.__init__ ends with four Pool-engine const memsets and an
    # all-engine barrier.  The barrier costs ~3.4us of kernel time because
    # every engine waits for the slow Q7 memsets before starting; nothing
    # here consumes those consts (the exp bias is built on ACT), so drop
    # the barrier (keep the memsets) and let the DMA stream start
    # immediately.
    preamble_barrier = [
        i.name
        for f in nc.m.functions
        for b in f.blocks
        for i in b.instructions
        if isinstance(i, (mybir.InstDrain, mybir.InstEventSemaphore))
    ]
    memT_d = nc.dram_tensor("memT", [128, DC * LSH], fp8, kind="ExternalInput")
    memn_d = nc.dram_tensor("memn", [128, LT * MD], fp8, kind="ExternalInput")
    # wt padded to 64 columns per chunk: dual-fp8 ldweights requires >=64
    # active PE columns (walrus 's3_lw_dual_fp8_restrictions').
    wt_d = nc.dram_tensor("wt", [128, (DC // 2) * 128], fp8, kind="ExternalInput")
    ctx_d = nc.dram_tensor("ctx", [NH, MD], f32, kind="ExternalOutput")
    s_d = nc.dram_tensor("s", [NH, 4], f32, kind="ExternalOutput")
    eye_np = np.zeros((128, NH), dtype=np.float16)
    for j in range(4):
        eye_np[32 * j : 32 * j + NH] = np.eye(NH, dtype=np.float16)
    eye_d = nc.inline_tensor(eye_np, "eye8")

    with tile.TileContext(nc) as tc:
        with (
            tc.tile_pool(name="const", bufs=1) as constp,
            tc.tile_pool(name="memTp", bufs=1) as memTp,
            tc.tile_pool(name="memnp", bufs=1) as memnp,
            tc.tile_pool(name="small", bufs=1) as smallp,
            tc.tile_pool(name="pssc", bufs=1, space=bass.MemorySpace.PSUM) as pssc,
            tc.tile_pool(name="pstr", bufs=1, space=bass.MemorySpace.PSUM) as pstr,
        ):
            # Small operands on the scalar (ACT) HWDGE queue.
            wt_sb = constp.tile([128, (DC // 2) * 128], fp8, tag="wt")
            nc.scalar.dma_start(out=wt_sb[:], in_=wt_d[:])
            eye_sb = constp.tile([128, NH], fp16, tag="eye")
            nc.scalar.dma_start(out=eye_sb[:], in_=eye_d[:])

            # Input stream on the sync HWDGE queue, memT strictly first.
            memT_sb = []
            memT_start = []
            pos = 0
            for g, gsz in enumerate(MEMT_GRPS):
                t_ = memTp.tile([128, gsz * LSH], fp8, tag=f"memT{g}")
                nc.sync.dma_start(
                    out=t_[:], in_=memT_d[:, pos * LSH : (pos + gsz) * LSH]
                )
                memT_sb.append(t_)
                memT_start.append(pos)
                pos += gsz

            def memT_pair(c2, nb):
                # [128, 2, 512] AP over chunks (2*c2, 2*c2+1), l-block nb
                c = 2 * c2
                for g in range(len(MEMT_GRPS) - 1, -1, -1):
                    if c >= memT_start[g]:
                        off = (c - memT_start[g]) * LSH
                        pair = memT_sb[g][:, off : off + 2 * LSH].rearrange(
                            "p (k l) -> p k l", k=2
                        )
                        return pair[:, :, nb * 512 : (nb + 1) * 512]
                raise AssertionError

            memn_sb = []
            memn_start = []
            pos = 0
            for k, gsz in enumerate(MEMN_GRPS):
                t_ = memnp.tile([128, gsz * MD], fp8, tag=f"memn{k}")
                nc.sync.dma_start(
                    out=t_[:], in_=memn_d[:, pos * MD : (pos + gsz) * MD]
                )
                memn_sb.append(t_)
                memn_start.append(pos)
                pos += gsz
            # Tiles 14/15: one pair tile fed by two small DMAs so the very
            # last transfer is 256 KiB and gates only four pair-matmuls.
            last_t = memnp.tile([128, 2 * MD], fp8, tag="memnL")
            nc.sync.dma_start(out=last_t[:, 0:MD], in_=memn_d[:, 14 * MD : 15 * MD])
            nc.sync.dma_start(out=last_t[:, MD:], in_=memn_d[:, 15 * MD : 16 * MD])
            memn_sb.append(last_t)
            memn_start.append(14)

            def memn_pair(t2, q):
                # [128, 2, 512] AP over l-tiles (2*t2, 2*t2+1), d-block q
                t = 2 * t2
                for k in range(len(memn_sb) - 1, -1, -1):
                    if t >= memn_start[k]:
                        off = (t - memn_start[k]) * MD
                        pair = memn_sb[k][:, off : off + 2 * MD].rearrange(
                            "p (k f) -> p k f", k=2
                        )
                        return pair[:, :, q * 512 : (q + 1) * 512]
                raise AssertionError

            def wt_pair(c2):
                return wt_sb[:, c2 * 128 : (c2 + 1) * 128].rearrange(
                    "p (k n) -> p k n", k=2
                )

            # Pass A: scoresT[n, l] = sum_d w[d, n] * memT[d, l].  fp8
            # DoubleRow: each matmul contracts 256 d (a chunk pair).  The
            # stationary is 64 columns (heads 0:8 real, rest zero) — dual
            # fp8 requires >=64 active columns AND psum partition base 0,
            # so all four l-block chains write rows 0:64 at position (0,0),
            # as the left/right halves of two 2-bank psum tiles; rows 8:64
            # are zeros and never read.  Issue order alternates tiles.
            scW1 = pssc.tile([64, 1024], f32, tag="scW1")
            scW2 = pssc.tile([64, 1024], f32, tag="scW2")
            sc_out = [
                scW1[:, 0:512],
                scW2[:, 0:512],
                scW1[:, 512:1024],
                scW2[:, 512:1024],
            ]
            sc_nb = [0, 2, 1, 3]   # l-block fed by each chain slot
            for c2 in range(DC // 2):
                for slot in range(4):
                    nc.tensor.matmul(
                        sc_out[slot],
                        wt_pair(c2),
                        memT_pair(c2, sc_nb[slot]),
                        start=(c2 == 0),
                        stop=(c2 == DC // 2 - 1),
                        perf_mode=DR,
                        tile_position=(0, 0),
                    )

            # p = exp(scores / WT_SCALE); accum_out gives the softmax
            # partial sum.  Four narrow exps so each band's transposes can
            # run on the PE while ACT computes the next exp.  The zero bias
            # is built on ACT itself (eye * 0.0) so nothing depends on the
            # stripped init barrier.
            zero_b = constp.tile([128, 1], f32, tag="zerob")
            nc.scalar.mul(zero_b[:], eye_sb[:, 0:1], 0.0)
            pT_sb = smallp.tile([NH, 2048], fp16, tag="pT")
            s_sb = smallp.tile([NH, 4], f32, tag="s")
            exp_src = [
                scW1[0:NH, 0:512],
                scW1[0:NH, 512:1024],
                scW2[0:NH, 0:512],
                scW2[0:NH, 512:1024],
            ]
            # exp block eb covers l-block: srcs map to blocks 0,1,2,3
            exp_blk = [0, 1, 2, 3]
            for eb in range(4):
                b = exp_blk[eb]
                nc.scalar.activation(
                    pT_sb[:, b * 512 : (b + 1) * 512], exp_src[eb], Exp,
                    bias=zero_b[0:NH, :],
                    scale=1.0 / WT_SCALE,
                    accum_out=s_sb[:, eb : eb + 1],
                )
            # Ship s mid-stream on the scalar queue.
            nc.scalar.dma_start(out=s_d[:], in_=s_sb[:])

            # Transpose p (8, L) -> per-l-tile (128, 8) stationary operands.
            # Band order matches exp completion so PE transposes overlap the
            # remaining ACT exps.
            tr_ps = pstr.tile([128, LT * NH], fp16, tag="tr")
            for t in range(LT):
                nc.tensor.transpose(
                    tr_ps[:, t * NH : (t + 1) * NH],
                    pT_sb[:, t * 128 : (t + 1) * 128],
                    eye_sb[0:NH, :],
                    tile_position=(0, 0),
                )
            # p padded to 64 columns per l-tile for the dual-fp8 ldweights;
            # columns 8:64 are whatever was in SBUF — they only feed psum
            # rows 8:64 / 72:128, which are never read.
            p_all = smallp.tile([128, LT * 64], fp8, tag="pall")
            nc.vector.tensor_copy(
                p_all[:].rearrange("p (t n) -> p t n", n=64)[:, :, 0:NH],
                tr_ps[:].rearrange("p (t n) -> p t n", n=NH),
            )

            def p_pair(t2):
                return p_all[:, t2 * 128 : (t2 + 1) * 128].rearrange(
                    "p (k n) -> p k n", k=2
                )

            # Pass B: ctx[n, d] = sum_l p[l, n] * mem[l, d].  fp8 DoubleRow
            # over l-tile pairs, t outer so accumulation rides the memn
            # DMAs.  Reuses the scores psum quadrants (WAR on the exps is
            # long satisfied); same two column groups, alternating.
            cx_q = [0, 2, 1, 3]   # d-block fed by each chain slot
            for t2 in range(LT // 2):
                for slot in range(4):
                    nc.tensor.matmul(
                        sc_out[slot],
                        p_pair(t2),
                        memn_pair(t2, cx_q[slot]),
                        start=(t2 == 0),
                        stop=(t2 == LT // 2 - 1),
                        perf_mode=DR,
                        tile_position=(0, 0),
                    )

            # Drain ctx to SBUF with ACT and DVE in parallel (one wide copy
            # each), then ship on both HWDGE queues (sync is idle once the
            # input stream ends).  scW1 rows 0:8 hold d 0:1024, scW2 rows
            # 0:8 hold d 1024:2048.
            ctx_lo = smallp.tile([NH, 1024], f32, tag="ctxlo")
            ctx_hi = smallp.tile([NH, 1024], f32, tag="ctxhi")
            nc.scalar.copy(ctx_lo[:], scW1[0:NH, :])
            nc.vector.tensor_copy(ctx_hi[:], scW2[0:NH, :])
            nc.sync.dma_start(out=ctx_d[:, 0:1024], in_=ctx_lo[:])
            nc.scalar.dma_start(out=ctx_d[:, 1024:], in_=ctx_hi[:])

    names = set(preamble_barrier)
    for f in nc.m.functions:
        for b in f.blocks:
            insts = b.instructions
            keep = [i for i in insts if i.name not in names]
            if len(keep) != len(insts):
                insts[:] = keep

    _split_multiwait(nc, mybir)
    nc.finalize()
    return nc


def _split_multiwait(nc, mybir):
    """Split instructions carrying >1 semaphore wait into single-wait NoOps.

    The walrus build in this environment encodes exactly one sync wait per
    engine instruction (setupSyncWait raises "Too many sync wait commands"
    otherwise), but Tile attaches the full wait set of the kernel-tail drain
    to one instruction.  Hoist all but the last wait onto dedicated NoOps on
    the same engine queue, which preserves semantics exactly.
    """
    k = 0
    for func in nc.m.functions:
        for block in func.blocks:
            insts = block.instructions
            i = 0
            while i < len(insts):
                inst = insts[i]
                si = inst.sync_info
                if si is not None and si.on_wait and len(si.on_wait) > 1:
                    waits = list(si.on_wait)
                    nops = []
                    for w in waits[:-1]:
                        nop = mybir.InstNoOp(
                            name=f"I-waitsplit-{k}",
                            engine=inst.engine,
                            bass_nofuse=True,
                            sync_info=mybir.SyncInfo(on_wait=[w], on_update=[]),
                        )
                        k += 1
                        nc.register_instruction(nop)
                        nops.append(nop)
                    inst.sync_info = mybir.SyncInfo(
                        on_wait=[waits[-1]], on_update=list(si.on_update)
                    )
                    insts[i:i] = nops
                    i += len(nops)
                i += 1


def _get_nc():
    if "nc" not in _CACHE:
        _CACHE["nc"] = _build_nc()
    return _CACHE["nc"]


def _host_prep(inputs):
    x = np.asarray(inputs["x"], dtype=np.float32).reshape(-1)          # (1024,)
    memory = np.asarray(inputs["memory"], dtype=np.float32)            # (L, MD)
    Wq = np.asarray(inputs["Wq"], dtype=np.float32)
    bq = np.asarray(inputs["bq"], dtype=np.float32)
    Wk = np.asarray(inputs["Wk"], dtype=np.float32)

    q = (x @ Wq.T + bq) * (DHEAD ** -0.5)                              # (1024,)
    # w[:, n] = sum_i q[i*8+n] * Wk[i*8+n, :]
    wmat = np.einsum(
        "in,ind->dn", q.reshape(DHEAD, NH), Wk.reshape(DHEAD, NH, MD),
        optimize=True,
    ).astype(np.float32)                                               # (MD, 8)

    import ml_dtypes
    fp8 = ml_dtypes.float8_e4m3
    # [p, c2*128 + k*64 + n] = w[(2*c2+k)*128 + p, n] * WT_SCALE, n<8; 0 pad
    wt64 = np.zeros((DC, 128, 64), dtype=np.float32)
    wt64[:, :, :NH] = (wmat * WT_SCALE).reshape(DC, 128, NH)
    wt_packed = np.ascontiguousarray(
        wt64.reshape(DC // 2, 2, 128, 64).transpose(2, 0, 1, 3)
        .reshape(128, (DC // 2) * 128)
    ).astype(fp8)
    in_maps = []
    for c in range(NCORES):
        shard = memory[c * LSH : (c + 1) * LSH]                        # (LSH, MD)
        # memT packed: [p, c*LSH + l] = shard[l, c*128+p]
        mt = shard.T.astype(fp8)                                       # (MD, LSH)
        memT_pack = np.ascontiguousarray(
            mt.reshape(DC, 128, LSH).transpose(1, 0, 2).reshape(128, DC * LSH)
        )
        # memn packed: [p, t*MD + d] = shard[t*128+p, d]
        mn = shard.astype(fp8)                                         # (LSH, MD)
        memn_pack = np.ascontiguousarray(
            mn.reshape(LT, 128, MD).transpose(1, 0, 2).reshape(128, LT * MD)
        )
        in_maps.append(
            {"memT": memT_pack, "memn": memn_pack, "wt": wt_packed}
        )
    return in_maps


def _host_finish(inputs, ctx_tot, s_tot):
    x = np.asarray(inputs["x"], dtype=np.float32).reshape(-1)
    Wv = np.asarray(inputs["Wv"], dtype=np.float32)
    bv = np.asarray(inputs["bv"], dtype=np.float32)
    Wo = np.asarray(inputs["Wo"], dtype=np.float32)
    bo = np.asarray(inputs["bo"], dtype=np.float32)

    ctx_norm = ctx_tot / s_tot                                         # (8, MD)
    feat_full = ctx_norm @ Wv.T + bv                                   # (8, 1024)
    feat = np.empty(H, dtype=np.float32)
    for n in range(NH):
        feat[n::NH] = feat_full[n, n::NH]
    ax = np.concatenate([x, feat])
    out = np.maximum(ax @ Wo.T + bo, 0.0).astype(np.float32)
    return out.reshape(1, 1, H)


def _run(inputs, trace=False, **spmd_kwargs):
    from concourse.bass_utils import run_bass_kernel_spmd

    nc = _get_nc()
    in_maps = _host_prep(inputs)
    res = run_bass_kernel_spmd(
        nc, in_maps, list(range(NCORES)), trace=trace, **spmd_kwargs
    )
    ctx_tot = np.zeros((NH, MD), dtype=np.float32)
    s_tot = np.zeros((NH, 1), dtype=np.float32)
    for r in res.results:
        ctx_tot += r["ctx"].astype(np.float32)
        s_tot += r["s"].astype(np.float32).sum(axis=1, keepdims=True)
    return _host_finish(inputs, ctx_tot, s_tot), res


def kernel(**inputs) -> np.ndarray:
    out, _ = _run(inputs, trace=False)
    return out


# revision 25
# speedup vs baseline: 1.3455x; 1.0987x over previous
"""Trainium2 Bass kernel for nn_Attention_82781199663345 (sparse_attention).

Reference computation (see problem statement):
    q  = x @ Wq.T + bq                    -> heads interleaved: head n owns q[i*8+n]
    K  = (memory @ Wk.T + bk)             -> (L, H), same interleave
    QK[n,l] = (d**-.5) * sum_i q[i*8+n] * K[l, i*8+n]
    attn = softmax_l(QK)                  (pad-mask term is exactly 0.0 in fp32)
    V  = memory @ Wv.T + bv
    feat[n,i] = sum_l attn[n,l] * V[l, i*8+n]
    out = relu(concat(x, feat) @ Wo.T + bo)

Algebraic refactor (exact in real arithmetic):
  * QK[n,l] = memory[l] . w_n + c_n   with  w_n = sum_i q_s[i*8+n] * Wk[i*8+n, :]
    (c_n is constant per head -> cancels in softmax, dropped)
  * sum_l attn[n,l] = 1  =>  feat row n = (attn[n] @ memory) @ Wv.T + bv, sliced
    at columns i*8+n.
  So the only L-sized (memory-bound) work is:
      scores = memory @ W            (L, 8)
      ctx    = softmax(scores).T @ memory   (8, 2048)
  Everything else is O(H*MD) and done on host in fp32.

Device strategy (8 cores, sequence-parallel over L):
  Each core gets its 2048-row shard twice in fp8e4m3: pre-transposed (d,l)
  for the scores pass and natural (l,d) for the context pass (the PE
  contracts over the partition dim only).  Softmax uses no max-subtraction:
  the final ctx/s division cancels any constant factor, and scores are
  O(+-2.5) so exp(scores) is far from fp16 overflow.  The cross-core
  combine is a pure sum on host: ctx = sum_c ctx_c, s = sum_c s_c.

Performance structure (from trace analysis):
  * Host packs both mem copies so every DMA moves [128, N] with N
    contiguous bytes per partition (8 KiB rows -> 128 descriptors/MiB);
    this sustains ~420 GB/s per core vs ~300 with 2 KiB lines.
  * All input DMAs ride the sync HWDGE queue (memT first, memn tapered);
    wt/eye/s/ctx_hi ride the scalar queue so they never stall the stream.
  * Matmuls: fp8e4 DoubleRow perf mode (256-deep contraction per pass),
    wt prescaled by 256 into fp8 (compensated in the exp scale), attention
    weights quantized to fp8 for the context pass.
  * Two PE column groups (rows 0:8 / 32:40), matmul order alternates
    groups: back-to-back same-group matmuls serialize (~227 ns/512 cols),
    alternating groups pipeline (~117 ns).
  * Scores psum is two 2-bank tiles, so softmax is 4 narrow exps whose
    band transposes pipeline on the PE while the next exp runs on ACT.
"""

import sys

import numpy as np

if "/opt/trn_rl_repo" not in sys.path:
    sys.path.insert(0, "/opt/trn_rl_repo")

H = 1024          # hidden dim
MD = 2048         # memory dim
L = 16384         # memory length
NH = 8            # heads
NCORES = 8
LSH = L // NCORES         # 2048 rows per core
DHEAD = H // NH           # 128
DC = MD // 128            # 16 contraction chunks (scores pass)
LT = LSH // 128           # 16 l-tiles (context pass)
WT_SCALE = 256.0          # wt prescale so fp8e4m3 stays in normal range
NB = 4                    # l-blocks of 512 (pass-A chains / memT DMA groups)
MEMN_GRPS = (4, 4, 4, 2, 2)      # memn l-tiles per DMA (pair-aligned tail)

_CACHE = {}


def _build_nc():
    import concourse.bass as bass
    import concourse.mybir as mybir
    from concourse import tile

    fp16 = mybir.dt.float16
    fp8 = mybir.dt.float8e4
    f32 = mybir.dt.float32
    Exp = mybir.ActivationFunctionType.Exp
    DR = mybir.MatmulPerfMode.DoubleRow

    nc = bass.Bass()
    # Bass.__init__ ends with four Pool-engine const memsets and an
    # all-engine barrier.  The barrier costs ~3.4us of kernel time because
    # every engine waits for the slow Q7 memsets before starting; nothing
    # here consumes those consts (the exp bias is built on ACT), so drop
    # the barrier (keep the memsets) and let the DMA stream start
    # immediately.
    preamble_barrier = [
        i.name
        for f in nc.m.functions
        for b in f.blocks
        for i in b.instructions
        if isinstance(i, (mybir.InstDrain, mybir.InstEventSemaphore))
    ]
    # memT is packed l-block-major: group b holds ALL d-chunks for l-block
    # b, so pass-A chain b (and its exp + p transposes) completes while the
    # rest of the stream is still in flight — the softmax pipeline hides
    # entirely inside the DMA window.
    memT_d = nc.dram_tensor("memT", [128, DC * LSH], fp8, kind="ExternalInput")
    memn_d = nc.dram_tensor("memn", [128, LT * MD], fp8, kind="ExternalInput")
    # wt padded to 64 columns per chunk: dual-fp8 ldweights requires >=64
    # active PE columns (walrus 's3_lw_dual_fp8_restrictions').
    wt_d = nc.dram_tensor("wt", [128, (DC // 2) * 128], fp8, kind="ExternalInput")
    ctx_d = nc.dram_tensor("ctx", [NH, MD], fp16, kind="ExternalOutput")
    s_d = nc.dram_tensor("s", [NH, 4], f32, kind="ExternalOutput")
    eye_np = np.zeros((128, NH), dtype=np.float16)
    for j in range(4):
        eye_np[32 * j : 32 * j + NH] = np.eye(NH, dtype=np.float16)
    eye_d = nc.inline_tensor(eye_np, "eye8")

    with tile.TileContext(nc) as tc:
        with (
            tc.tile_pool(name="const", bufs=1) as constp,
            tc.tile_pool(name="memTp", bufs=1) as memTp,
            tc.tile_pool(name="memnp", bufs=1) as memnp,
            tc.tile_pool(name="small", bufs=1) as smallp,
            tc.tile_pool(name="pssc", bufs=1, space=bass.MemorySpace.PSUM) as pssc,
            tc.tile_pool(name="pstr", bufs=1, space=bass.MemorySpace.PSUM) as pstr,
        ):
            # Small operands on the scalar (ACT) HWDGE queue.
            wt_sb = constp.tile([128, (DC // 2) * 128], fp8, tag="wt")
            nc.scalar.dma_start(out=wt_sb[:], in_=wt_d[:])
            eye_sb = constp.tile([128, NH], fp16, tag="eye")
            nc.scalar.dma_start(out=eye_sb[:], in_=eye_d[:])

            # Input stream on the sync HWDGE queue, memT strictly first.
            # One 1 MiB DMA per l-block (all 16 d-chunks for that block).
            memT_sb = []
            for b in range(NB):
                t_ = memTp.tile([128, DC * 512], fp8, tag=f"memT{b}")
                nc.sync.dma_start(
                    out=t_[:], in_=memT_d[:, b * DC * 512 : (b + 1) * DC * 512]
                )
                memT_sb.append(t_)

            def memT_pair(b, c2):
                # [128, 2, 512] AP over chunks (2*c2, 2*c2+1) of l-block b
                return memT_sb[b][:, c2 * 1024 : (c2 + 1) * 1024].rearrange(
                    "p (k l) -> p k l", k=2
                )

            memn_sb = []
            memn_start = []
            pos = 0
            for k, gsz in enumerate(MEMN_GRPS):
                t_ = memnp.tile([128, gsz * MD], fp8, tag=f"memn{k}")
                nc.sync.dma_start(
                    out=t_[:], in_=memn_d[:, pos * MD : (pos + gsz) * MD]
                )
                memn_sb.append(t_)
                memn_start.append(pos)
                pos += gsz

            def memn_pair(t2, q):
                # [128, 2, 512] AP over l-tiles (2*t2, 2*t2+1), d-block q
                t = 2 * t2
                for k in range(len(memn_sb) - 1, -1, -1):
                    if t >= memn_start[k]:
                        off = (t - memn_start[k]) * MD
                        pair = memn_sb[k][:, off : off + 2 * MD].rearrange(
                            "p (k f) -> p k f", k=2
                        )
                        return pair[:, :, q * 512 : (q + 1) * 512]
                raise AssertionError

            # Interleaved PE program order: chain b's matmuls, then (once
            # exp b-1 has run on ACT in parallel) the transposes of band
            # b-1 fill the DMA group-boundary wait.
            pe_sched = []
            for b in range(NB):
                pe_sched.append(("chain", b))
                if b >= 1:
                    pe_sched.append(("trs", b - 1))
            pe_sched.append(("trs", NB - 1))

            def wt_pair(c2):
                return wt_sb[:, c2 * 128 : (c2 + 1) * 128].rearrange(
                    "p (k n) -> p k n", k=2
                )

            # Pass A: scoresT[n, l] = sum_d w[d, n] * memT[d, l].  fp8
            # DoubleRow: each matmul contracts 256 d (a chunk pair).  The
            # stationary is 64 columns (heads 0:8 real, rest zero) — dual
            # fp8 requires >=64 active columns AND psum partition base 0,
            # so all four l-block chains write rows 0:64 at position (0,0),
            # as the left/right halves of two 2-bank psum tiles; rows 8:64
            # are zeros and never read.
            #
            # Chain b runs as one block of 8 pair-matmuls gated only on
            # memT group b; exp b (ACT) runs while chain b+1 streams, and
            # band b's p-transposes fill the next group-boundary wait on
            # the PE.  Everything softmax-related hides inside the DMA
            # window.
            scW1 = pssc.tile([64, 1024], f32, tag="scW1")
            scW2 = pssc.tile([64, 1024], f32, tag="scW2")
            sc_out = [
                scW1[:, 0:512],
                scW1[:, 512:1024],
                scW2[:, 0:512],
                scW2[:, 512:1024],
            ]

            zero_b = constp.tile([128, 1], f32, tag="zerob")
            nc.scalar.mul(zero_b[:], eye_sb[:, 0:1], 0.0)
            pT_sb = smallp.tile([NH, 2048], fp16, tag="pT")
            s_sb = smallp.tile([NH, 4], f32, tag="s")
            tr_ps = pstr.tile([128, LT * NH], fp16, tag="tr")

            def emit_chain(b):
                for c2 in range(DC // 2):
                    nc.tensor.matmul(
                        sc_out[b],
                        wt_pair(c2),
                        memT_pair(b, c2),
                        start=(c2 == 0),
                        stop=(c2 == DC // 2 - 1),
                        perf_mode=DR,
                        tile_position=(0, 0),
                    )

            def emit_exp(b):
                nc.scalar.activation(
                    pT_sb[:, b * 512 : (b + 1) * 512],
                    sc_out[b][0:NH, :], Exp,
                    bias=zero_b[0:NH, :],
                    scale=1.0 / WT_SCALE,
                    accum_out=s_sb[:, b : b + 1],
                )

            def emit_trs(b):
                for t in range(4 * b, 4 * b + 4):
                    nc.tensor.transpose(
                        tr_ps[:, t * NH : (t + 1) * NH],
                        pT_sb[:, t * 128 : (t + 1) * 128],
                        eye_sb[0:NH, :],
                        tile_position=(0, 0),
                    )

            for b in range(NB):
                emit_chain(b)
                emit_exp(b)
                if b >= 1:
                    emit_trs(b - 1)
            emit_trs(NB - 1)
            # Ship s mid-stream on the scalar queue.
            nc.scalar.dma_start(out=s_d[:], in_=s_sb[:])
            # p padded to 64 columns per l-tile for the dual-fp8 ldweights;
            # columns 8:64 are whatever was in SBUF — they only feed psum
            # rows 8:64 / 72:128, which are never read.
            p_all = smallp.tile([128, LT * 64], fp8, tag="pall")
            nc.vector.tensor_copy(
                p_all[:].rearrange("p (t n) -> p t n", n=64)[:, :, 0:NH],
                tr_ps[:].rearrange("p (t n) -> p t n", n=NH),
            )

            def p_pair(t2):
                return p_all[:, t2 * 128 : (t2 + 1) * 128].rearrange(
                    "p (k n) -> p k n", k=2
                )

            # Pass B: ctx[n, d] = sum_l p[l, n] * mem[l, d].  fp8 DoubleRow
            # over l-tile pairs, t outer so accumulation rides the memn
            # DMAs.  Reuses the scores psum quadrants (WAR on the exps is
            # long satisfied); same two column groups, alternating.
            for t2 in range(LT // 2):
                for q in range(4):
                    nc.tensor.matmul(
                        sc_out[q],
                        p_pair(t2),
                        memn_pair(t2, q),
                        start=(t2 == 0),
                        stop=(t2 == LT // 2 - 1),
                        perf_mode=DR,
                        tile_position=(0, 0),
                    )

            # Drain ctx to SBUF with ACT and DVE in parallel (one wide copy
            # each, cast to fp16 to halve the ship), then ship on both
            # HWDGE queues (sync is idle once the input stream ends).  scW1
            # rows 0:8 hold d 0:1024, scW2 rows 0:8 hold d 1024:2048.
            ctx_lo = smallp.tile([NH, 1024], fp16, tag="ctxlo")
            ctx_hi = smallp.tile([NH, 1024], fp16, tag="ctxhi")
            nc.scalar.copy(ctx_lo[:], scW1[0:NH, :])
            nc.vector.tensor_copy(ctx_hi[:], scW2[0:NH, :])
            nc.sync.dma_start(out=ctx_d[:, 0:1024], in_=ctx_lo[:])
            nc.scalar.dma_start(out=ctx_d[:, 1024:], in_=ctx_hi[:])

    names = set(preamble_barrier)
    for f in nc.m.functions:
        for b in f.blocks:
            insts = b.instructions
            keep = [i for i in insts if i.name not in names]
            if len(keep) != len(insts):
                insts[:] = keep

    _split_multiwait(nc, mybir)
    nc.finalize()
    return nc


def _split_multiwait(nc, mybir):
    """Split instructions carrying >1 semaphore wait into single-wait NoOps.

    The walrus build in this environment encodes exactly one sync wait per
    engine instruction (setupSyncWait raises "Too many sync wait commands"
    otherwise), but Tile attaches the full wait set of the kernel-tail drain
    to one instruction.  Hoist all but the last wait onto dedicated NoOps on
    the same engine queue, which preserves semantics exactly.
    """
    k = 0
    for func in nc.m.functions:
        for block in func.blocks:
            insts = block.instructions
            i = 0
            while i < len(insts):
                inst = insts[i]
                si = inst.sync_info
                if si is not None and si.on_wait and len(si.on_wait) > 1:
                    waits = list(si.on_wait)
                    nops = []
                    for w in waits[:-1]:
                        nop = mybir.InstNoOp(
                            name=f"I-waitsplit-{k}",
                            engine=inst.engine,
                            bass_nofuse=True,
                            sync_info=mybir.SyncInfo(on_wait=[w], on_update=[]),
                        )
                        k += 1
                        nc.register_instruction(nop)
                        nops.append(nop)
                    inst.sync_info = mybir.SyncInfo(
                        on_wait=[waits[-1]], on_update=list(si.on_update)
                    )
                    insts[i:i] = nops
                    i += len(nops)
                i += 1


def _get_nc():
    if "nc" not in _CACHE:
        _CACHE["nc"] = _build_nc()
    return _CACHE["nc"]


def _host_prep(inputs):
    x = np.asarray(inputs["x"], dtype=np.float32).reshape(-1)          # (1024,)
    memory = np.asarray(inputs["memory"], dtype=np.float32)            # (L, MD)
    Wq = np.asarray(inputs["Wq"], dtype=np.float32)
    bq = np.asarray(inputs["bq"], dtype=np.float32)
    Wk = np.asarray(inputs["Wk"], dtype=np.float32)

    q = (x @ Wq.T + bq) * (DHEAD ** -0.5)                              # (1024,)
    # w[:, n] = sum_i q[i*8+n] * Wk[i*8+n, :]
    wmat = np.einsum(
        "in,ind->dn", q.reshape(DHEAD, NH), Wk.reshape(DHEAD, NH, MD),
        optimize=True,
    ).astype(np.float32)                                               # (MD, 8)

    import ml_dtypes
    fp8 = ml_dtypes.float8_e4m3
    # [p, c2*128 + k*64 + n] = w[(2*c2+k)*128 + p, n] * WT_SCALE, n<8; 0 pad
    wt64 = np.zeros((DC, 128, 64), dtype=np.float32)
    wt64[:, :, :NH] = (wmat * WT_SCALE).reshape(DC, 128, NH)
    wt_packed = np.ascontiguousarray(
        wt64.reshape(DC // 2, 2, 128, 64).transpose(2, 0, 1, 3)
        .reshape(128, (DC // 2) * 128)
    ).astype(fp8)
    in_maps = []
    for c in range(NCORES):
        shard = memory[c * LSH : (c + 1) * LSH]                        # (LSH, MD)
        # memT packed l-block-major: [p, b*8192 + cc*512 + l'] =
        #   shard[b*512 + l', cc*128 + p]
        mt = shard.T.astype(fp8)                                       # (MD, LSH)
        memT_pack = np.ascontiguousarray(
            mt.reshape(DC, 128, NB, 512).transpose(1, 2, 0, 3)
            .reshape(128, DC * LSH)
        )
        # memn packed: [p, t*MD + d] = shard[t*128+p, d]
        mn = shard.astype(fp8)                                         # (LSH, MD)
        memn_pack = np.ascontiguousarray(
            mn.reshape(LT, 128, MD).transpose(1, 0, 2).reshape(128, LT * MD)
        )
        in_maps.append(
            {"memT": memT_pack, "memn": memn_pack, "wt": wt_packed}
        )
    return in_maps


def _host_finish(inputs, ctx_tot, s_tot):
    x = np.asarray(inputs["x"], dtype=np.float32).reshape(-1)
    Wv = np.asarray(inputs["Wv"], dtype=np.float32)
    bv = np.asarray(inputs["bv"], dtype=np.float32)
    Wo = np.asarray(inputs["Wo"], dtype=np.float32)
    bo = np.asarray(inputs["bo"], dtype=np.float32)

    ctx_norm = ctx_tot / s_tot                                         # (8, MD)
    feat_full = ctx_norm @ Wv.T + bv                                   # (8, 1024)
    feat = np.empty(H, dtype=np.float32)
    for n in range(NH):
        feat[n::NH] = feat_full[n, n::NH]
    ax = np.concatenate([x, feat])
    out = np.maximum(ax @ Wo.T + bo, 0.0).astype(np.float32)
    return out.reshape(1, 1, H)


def _run(inputs, trace=False, **spmd_kwargs):
    from concourse.bass_utils import run_bass_kernel_spmd

    nc = _get_nc()
    in_maps = _host_prep(inputs)
    res = run_bass_kernel_spmd(
        nc, in_maps, list(range(NCORES)), trace=trace, **spmd_kwargs
    )
    ctx_tot = np.zeros((NH, MD), dtype=np.float32)
    s_tot = np.zeros((NH, 1), dtype=np.float32)
    for r in res.results:
        ctx_tot += r["ctx"].astype(np.float32)
        s_tot += r["s"].astype(np.float32).sum(axis=1, keepdims=True)
    return _host_finish(inputs, ctx_tot, s_tot), res


def kernel(**inputs) -> np.ndarray:
    out, _ = _run(inputs, trace=False)
    return out
